# revision 61
# baseline (speedup 1.0000x reference)
"""MQA attention kernel v26 (B=2, T=2048, C=2048, 16 query heads, D=128,
RoPE, causal) for 8 Trainium2 NeuronCores.

Sharding: core = (batch, head-group-of-4), partial output projections summed
on host.  ~253us median / ~251us best at the fast clock state, ~290us slow
state (vs 283/338us v4 baseline); rel err 6.4e-3.

v26: weight quarters q2/q3 ride the sync queue interleaved with the x
pairs in kc-need order, so the 3MB weight stream no longer trails the x
stream (chunk-0 window idle 8.2 -> 2.7us).

v22..24 theme — on in-order engines, emission position IS the schedule; emit
deferrable work last:
- c0 attention segments run BEFORE each section's rope work (the ~7us of
  rope vector-ops, not needed until the weave bands, were delaying the
  segments' mask/denominator chain); k-rope of chunks 1-3 deferred too.
- the v-transpose DMA issues AFTER the q-evac copies: its semaphore
  pre-wait otherwise parks the scalar engine ~8us between the k-evac and
  the q-evacs that gate the next section's B-pass PSUM banks.

v20 key fix: dma_start BLOCKS the issuing engine until the previous
transfer on the same queue-semaphore completes — so all x^T waves live on
the otherwise-idle sync queue (its in-order blocking IS the demand pacing)
and the scalar engine keeps only the 4 weight-quarter issues, leaving it
free for the PSUM evac copies that gate each phase-1 section boundary.

Structure (v5..v19 over the v4 baseline):
- DMA (v5..v19): host pre-arranges all tensors partition-major (>=1KB contiguous
  lines); x^T fully SBUF-resident via large demand-ordered DMAs (weight
  quarters interleaved with x t-chunk-0 columns in kc order, then chunk 1,
  then t-half B); fused wq|wk|wv dram tensor; Wo loaded at tcn=2; outputs
  staged per 128-row group, large row DMAs on rotating queues, per-quarter
  drain for the last 4 groups (gpsimd excluded there so its ~5us engine
  drain retires early).
- PE warmup matmuls on a memset tile cover the ~7us framework preamble +
  first input DMAs with the clock fully ramped.
- Unified tensor-filler list: the in-order PE queue never sits behind an
  exp-dependent matmul — output-projection [128,512] po-quarters (band
  c uses chunk c-1's m-groups) and the NEXT chunk's k/v kc-steps (A-pass)
  are drained between score pairs / after diagonal scores.  Phase-1 chunk 0
  keeps the dense 6-stream loop (it is DMA-paced).
- attn segments: 2-pair softmax lookahead (scores(p+2) before pv(p));
  denominator ones-matmuls deferred past the DVE add tree; v transposed via
  dma_start_transpose straight into the [keys,D] SBUF layout (no PSUM /
  tensor-engine transposes).
Known floor: ~203us of bf16 matmul rows at 2.4GHz; fp8 DoubleRow and
AllGather k/v-dedup were measured and ruled out (see memory notes).
"""

import os
import sys

if "/opt/trn_rl_repo" not in sys.path:
    sys.path.insert(0, "/opt/trn_rl_repo")

import numpy as np

import concourse.bacc as bacc
import concourse.mybir as mybir
import concourse.tile as tile
from concourse.bass_utils import run_bass_kernel_spmd

T = 2048
C = 2048
D = 128
N_HEAD = 16
HPC = 4
N_CORES = 8
F32 = mybir.dt.float32
BF16 = mybir.dt.bfloat16
EXP = mybir.ActivationFunctionType.Exp

MD = BF16


def build_program():
    nc = bacc.Bacc("TRN2", target_bir_lowering=False, debug=False)

    xt = nc.dram_tensor("xt", [C, T], MD, kind="ExternalInput")
    wqkv = nc.dram_tensor("wqkv", [128, 16, 768], MD, kind="ExternalInput")
    wo = nc.dram_tensor("wo", [128, 4, 2048], MD, kind="ExternalInput")
    css = nc.dram_tensor("css", [128, 2 * T], MD, kind="ExternalInput")
    out = nc.dram_tensor("out", [128, 16, 2048], BF16, kind="ExternalOutput")

    xt_r = xt.rearrange("(ko p) t -> p ko t", p=128)

    with (
        tile.TileContext(nc) as tc,
        tc.tile_pool(name="consts", bufs=1) as consts,
        tc.tile_pool(name="qkpool", bufs=5) as qkpool,
        tc.tile_pool(name="ytpool", bufs=4) as ytpool,
        tc.tile_pool(name="vttp", bufs=2) as vttp,
        tc.tile_pool(name="ptp", bufs=5) as ptpool,
        tc.tile_pool(name="otp", bufs=3) as otp,
        tc.tile_pool(name="swp", bufs=3) as swp,
        tc.tile_pool(name="pad", bufs=6) as pad,
        tc.tile_pool(name="bcp", bufs=3) as bcp,
        tc.tile_pool(name="psb", bufs=2, space="PSUM") as psb,
        tc.tile_pool(name="pss", bufs=4, space="PSUM") as pssm,
    ):
        # ---- input DMAs: big contiguous-line transfers, demand-ordered so
        # the ~315GB/s per-core DMA bandwidth goes to what compute needs
        # next.  Upfront: sync queue x^T t-half A (chunks 0-1), scalar queue
        # weight quarters, gpsimd cos/sin.  x^T t-half B is issued at tcn=1,
        # Wo at tcn=2 (see phase 1 loop). ----
        # PE warmup: dependency-free matmuls on a memset tile keep the PE
        # busy through the DMA wait so the clock is fully ramped (and the
        # pipeline hot) when the real accumulation chains start.
        warm_mv = consts.tile([128, 512], MD, tag="warm")
        nc.gpsimd.memset(warm_mv, 0.0)

        # input DMAs: interleave weight quarters and x^T pairs across the
        # sync/scalar queues so arrival order tracks the kc consumption
        # order of the first t-chunk.
        xts = consts.tile([128, 16, T], MD, tag="xts")
        wq_t = consts.tile([128, 16, 768], MD, tag="wq")

        def wq_dma(eng, q):
            eng.dma_start(
                out=wq_t[:, 4 * q : 4 * q + 4, :], in_=wqkv[:, 4 * q : 4 * q + 4, :]
            )

        def xt_dma(eng, kp, half):
            tsl = slice(0, 512)
            eng.dma_start(
                out=xts[:, 2 * kp : 2 * kp + 2, tsl],
                in_=xt_r[:, 2 * kp : 2 * kp + 2, tsl],
            )

        csst = consts.tile([128, 2 * T], MD, tag="css")
        nc.gpsimd.dma_start(out=csst, in_=css[:, :])
        # wave 0: weight quarters interleaved with t-chunk-0 columns only,
        # in kc order — chunk 0's consumption is DMA-paced, so nothing else
        # competes for bandwidth until its last kc tile has landed
        # x^T entirely on the sync queue: dma_start blocks the issuing
        # engine until the previous transfer on the same semaphore is done,
        # so the idle sync engine absorbs all the pacing stalls while the
        # scalar engine stays free for PSUM evac copies.
        # weight quarters q0/q1 on scalar; q2/q3 interleaved into the sync
        # queue in kc-need order — otherwise the 3MB weight stream trails
        # the 2MB x stream and q3's arrival (~26us) gates kc12-15
        wq_dma(nc.scalar, 0)
        wq_dma(nc.scalar, 1)
        for kp in range(4):
            xt_dma(nc.sync, kp, 0)
        wq_dma(nc.sync, 2)
        xt_dma(nc.sync, 4, 0)
        xt_dma(nc.sync, 5, 0)
        wq_dma(nc.sync, 3)
        xt_dma(nc.sync, 6, 0)
        xt_dma(nc.sync, 7, 0)
        for kp in range(8):
            nc.sync.dma_start(
                out=xts[:, 2 * kp : 2 * kp + 2, 512:1024],
                in_=xt_r[:, 2 * kp : 2 * kp + 2, 512:1024],
            )
        for kp in range(8):
            nc.sync.dma_start(
                out=xts[:, 2 * kp : 2 * kp + 2, 1024:2048],
                in_=xt_r[:, 2 * kp : 2 * kp + 2, 1024:2048],
            )
        for w in range(24):
            pw = pssm.tile([128, 512], F32, tag="small", name=f"warm{w}")
            nc.tensor.matmul(pw, warm_mv[:, 0:128], warm_mv, start=True, stop=True)

        wot = consts.tile([128, 4, 2048], MD, tag="wo")

        # on-chip constants: ones / causal-triangle
        ones = consts.tile([128, 128], MD, tag="ones")
        nc.gpsimd.memset(ones, 1.0)
        tri = consts.tile([128, 128], MD, tag="tri")
        nc.gpsimd.memset(tri, 1.0)
        nc.gpsimd.affine_select(
            out=tri,
            in_=tri,
            compare_op=mybir.AluOpType.is_ge,
            fill=0.0,
            base=0,
            pattern=[[1, 128]],
            channel_multiplier=-1,
        )

        qk = [qkpool.tile([128, T], MD, tag="qk", name=f"qk{i}") for i in range(5)]
        yt = [ytpool.tile([128, T], MD, tag="yt", name=f"yt{h}") for h in range(4)]
        vsb = [consts.tile([128, 8, 128], MD, tag=f"vsb{g}", name=f"vsb{g}") for g in range(2)]

        def vtile(j):
            return vsb[j // 8][:, j % 8, :]

        def wosl(h, cn):  # [128, 512] slice of Wo for output cols cn
            return wot[:, h, cn * 512 : (cn + 1) * 512]

        def ktile(j):
            return qk[4][:, j * 128 : (j + 1) * 128]

        # ---- phase 2 + 3 woven: attention per (chunk, head); the previous
        # chunk's output-projection emits as [128,512] po-quarter filler
        # INSIDE each segment (between score pairs), so the in-order tensor
        # queue always has exp-independent work while softmax runs ----
        filler = []

        def drain_filler(n):
            while n > 0 and filler:
                filler.pop(0)()
                n -= 1

        def attn_segment(c, h):
            if c == 0:
                # c0 segments have no pair loop: without this, their
                # diagonal scores (which wait on the pQ/pR bank evacs) sit
                # at the head of the tensor queue with no filler ahead.
                # ~12 A-pass steps (~5us) cover the measured evac wait.
                drain_filler(12)
            qsl = qk[h][:, c * 512 : (c + 1) * 512]
            py = pssm.tile([128, 512], F32, tag="small", name=f"py{c}_{h}")
            psm = pssm.tile([128, 512], F32, tag="small", name=f"psm{c}_{h}")
            py_on = False
            sm_on = False
            pend = None
            pend2 = None

            def emit_pv(pT, j0):
                nonlocal py_on
                nc.tensor.matmul(py, vtile(j0), pT[:, 0:512], start=not py_on, stop=False)
                py_on = True
                nc.tensor.matmul(py, vtile(j0 + 1), pT[:, 512:1024], start=False, stop=False)

            sm_src = []  # summed-exp tiles; their ones-matmuls are deferred
            # to the segment end so the tensor queue never stalls on the
            # DVE add tree.
            pvq = []  # two-pair lookahead: pv(p) is emitted after
            # scores(p+2), giving each exp ~2 score-pairs of tensor cover
            for p in range(2 * c):
                j0 = 2 * p
                pss = psb.tile([128, 1024], F32, tag="big", name=f"pss{c}_{h}_{p}")
                nc.tensor.matmul(pss[:, 0:512], ktile(j0), qsl, start=True, stop=True)
                nc.tensor.matmul(pss[:, 512:1024], ktile(j0 + 1), qsl, start=True, stop=True)
                if len(pvq) >= 2:
                    emit_pv(*pvq.pop(0))
                drain_filler(3 - c if c < 3 else 1)
                pT = ptpool.tile([128, 1024], MD, tag="pt", name=f"pt{c}_{h}_{p}")
                nc.scalar.activation(out=pT, in_=pss, func=EXP)
                pvq.append((pT, j0))
                padd = pad.tile([128, 512], MD, tag="padd", name=f"pa{c}_{h}_{p}")
                nc.vector.tensor_add(out=padd, in0=pT[:, 0:512], in1=pT[:, 512:1024])
                if pend is None:
                    pend = padd
                else:
                    qadd = pad.tile([128, 512], MD, tag="padd", name=f"qa{c}_{h}_{p}")
                    nc.vector.tensor_add(out=qadd, in0=pend, in1=padd)
                    pend = None
                    if pend2 is None:
                        pend2 = qadd
                    else:
                        oadd = pad.tile([128, 512], MD, tag="padd", name=f"oa{c}_{h}_{p}")
                        nc.vector.tensor_add(out=oadd, in0=pend2, in1=qadd)
                        sm_src.append(oadd)
                        pend2 = None
            if pend2 is not None:
                sm_src.append(pend2)
                pend2 = None
            # diagonal group: r0 [0:512] + r1 [512:896] in A; r2 [0:256] +
            # r3 [256:384] in B (both allocated up front: no exp stall)
            jb = 4 * c
            pdA = psb.tile([128, 1024], F32, tag="big", name=f"pdA{c}_{h}")
            pdB = psb.tile([128, 1024], F32, tag="big", name=f"pdB{c}_{h}")
            nc.tensor.matmul(pdA[:, 0:512], ktile(jb), qsl, start=True, stop=True)
            nc.tensor.matmul(
                pdA[:, 512:896],
                ktile(jb + 1),
                qk[h][:, c * 512 + 128 : (c + 1) * 512],
                start=True,
                stop=True,
            )
            nc.tensor.matmul(
                pdB[:, 0:256],
                ktile(jb + 2),
                qk[h][:, c * 512 + 256 : (c + 1) * 512],
                start=True,
                stop=True,
            )
            nc.tensor.matmul(
                pdB[:, 256:384],
                ktile(jb + 3),
                qk[h][:, c * 512 + 384 : (c + 1) * 512],
                start=True,
                stop=True,
            )
            while pvq:
                emit_pv(*pvq.pop(0))
            for oadd in sm_src:
                nc.tensor.matmul(psm, ones, oadd, start=not sm_on, stop=False)
                sm_on = True
            drain_filler(2)
            pTA = ptpool.tile([128, 1024], MD, tag="pt", name=f"ptA{c}_{h}")
            pTB = ptpool.tile([128, 1024], MD, tag="pt", name=f"ptB{c}_{h}")
            nc.scalar.activation(out=pTA[:, 0:896], in_=pdA[:, 0:896], func=EXP)
            nc.scalar.activation(out=pTB[:, 0:384], in_=pdB[:, 0:384], func=EXP)
            ve = nc.vector
            ve.tensor_mul(out=pTA[:, 0:128], in0=pTA[:, 0:128], in1=tri)
            ve.tensor_mul(out=pTA[:, 512:640], in0=pTA[:, 512:640], in1=tri)
            ve.tensor_mul(out=pTB[:, 0:128], in0=pTB[:, 0:128], in1=tri)
            ve.tensor_mul(out=pTB[:, 256:384], in0=pTB[:, 256:384], in1=tri)
            nc.tensor.matmul(py, vtile(jb), pTA[:, 0:512], start=not py_on, stop=False)
            nc.tensor.matmul(py[:, 128:512], vtile(jb + 1), pTA[:, 512:896], start=False, stop=False)
            nc.tensor.matmul(py[:, 256:512], vtile(jb + 2), pTB[:, 0:256], start=False, stop=False)
            nc.tensor.matmul(py[:, 384:512], vtile(jb + 3), pTB[:, 256:384], start=False, stop=True)
            # diagonal denominators collapse on DVE, then one ones-matmul
            pd = pad.tile([128, 512], MD, tag="padd", name=f"pd{c}_{h}")
            ve.tensor_copy(out=pd[:, 0:128], in_=pTA[:, 0:128])
            ve.tensor_add(out=pd[:, 128:512], in0=pTA[:, 128:512], in1=pTA[:, 512:896])
            ve.tensor_add(out=pd[:, 256:512], in0=pd[:, 256:512], in1=pTB[:, 0:256])
            ve.tensor_add(out=pd[:, 384:512], in0=pd[:, 384:512], in1=pTB[:, 256:384])
            nc.tensor.matmul(psm, ones, pd, start=not sm_on, stop=True)
            bc = bcp.tile([128, 512], F32, tag="bc", name=f"bc{c}_{h}")
            nc.vector.reciprocal_approx_fast(out=bc, in_=psm)
            nc.vector.tensor_mul(
                out=yt[h][:, c * 512 : (c + 1) * 512], in0=py, in1=bc
            )

        out_q = [nc.sync, nc.gpsimd, nc.scalar]
        otms = {}

        def make_quarter(m, cn, last=False):
            def q():
                if cn == 0:
                    otms[m] = otp.tile([128, 2048], MD, tag="ot", name=f"ot{m}")
                otm = otms[m]
                po = pssm.tile([128, 512], F32, tag="small", name=f"po{m}_{cn}")
                for h in range(4):
                    nc.tensor.matmul(
                        po,
                        yt[h][:, m * 128 : (m + 1) * 128],
                        wosl(h, cn),
                        start=h == 0,
                        stop=h == 3,
                    )
                osl = otm[:, cn * 512 : (cn + 1) * 512]
                if cn == 0 or cn == 2:
                    nc.vector.tensor_copy(out=osl, in_=po)
                else:
                    nc.scalar.copy(out=osl, in_=po)
                if last:  # drain each quarter immediately; avoid gpsimd so
                    # its ~5us engine drain retires before the last compute
                    (nc.sync if (m + cn) % 2 else nc.scalar).dma_start(
                        out=out[:, m, cn * 512 : (cn + 1) * 512], in_=osl
                    )
                elif cn == 3:
                    out_q[m % 3].dma_start(out=out[:, m, :], in_=otm)
                if cn == 3:
                    del otms[m]
            return q

        def p3_mgroup(m, last=False):
            for cn in range(4):
                make_quarter(m, cn, last)()

        # ---- phase 1: q/k/v projections, t-chunk-major, split per chunk
        # into an A-pass (k/v) and a B-pass (q).  The NEXT chunk's A-pass
        # kc-steps are queued as filler so the c0 attention segments woven
        # into each chunk's tail never leave the tensor engine idle. ----
        attn_after = {1: [0], 2: [1, 2], 3: [3]}  # tcn -> c0 heads to emit

        def rope(o, tcn):
            tsl = slice(512 * tcn, 512 * (tcn + 1))
            qc = qk[o]
            sw = swp.tile([128, 512], MD, tag="sw", name=f"sw{tcn}_{o}")
            nc.gpsimd.dma_start(out=sw[0:64, :], in_=qc[64:128, tsl])
            nc.gpsimd.dma_start(out=sw[64:128, :], in_=qc[0:64, tsl])
            nc.vector.tensor_mul(out=qc[:, tsl], in0=qc[:, tsl], in1=csst[:, tsl])
            eng = nc.gpsimd if o in (1, 2) else nc.vector
            eng.tensor_mul(
                out=sw[:], in0=sw[:], in1=csst[:, T + 512 * tcn : T + 512 * (tcn + 1)]
            )
            nc.vector.tensor_add(out=qc[:, tsl], in0=qc[:, tsl], in1=sw[:])

        def make_a_steps(tcn):
            tsl = slice(512 * tcn, 512 * (tcn + 1))
            pk = pssm.tile([128, 512], F32, tag="small", name=f"pk{tcn}")
            pv = pssm.tile([128, 512], F32, tag="small", name=f"pv{tcn}")

            def step(kc):
                def f():
                    xtt = xts[:, kc, tsl]
                    st, sp = kc == 0, kc == 15
                    nc.tensor.matmul(pk, wq_t[:, kc, 512:640], xtt, start=st, stop=sp)
                    nc.tensor.matmul(pv, wq_t[:, kc, 640:768], xtt, start=st, stop=sp)
                return f

            return pk, pv, [step(kc) for kc in range(16)]

        kvt = None
        for tcn in range(4):
            if tcn == 2:  # Wo: needed from the first p3_mgroup
                nc.sync.dma_start(out=wot[:, :, 0:1024], in_=wo[:, :, 0:1024])
                nc.sync.dma_start(out=wot[:, :, 1024:2048], in_=wo[:, :, 1024:2048])
            tsl = slice(512 * tcn, 512 * (tcn + 1))
            pQ = psb.tile([128, 1024], F32, tag="big", name=f"pQ{tcn}")  # q0|q1
            pR = psb.tile([128, 1024], F32, tag="big", name=f"pR{tcn}")  # q2|q3
            if tcn == 0:
                # chunk 0 is DMA-paced: keep the dense combined loop so
                # every arriving kc tile feeds 6 matmuls at once
                pk = pssm.tile([128, 512], F32, tag="small", name="pk0")
                pv = pssm.tile([128, 512], F32, tag="small", name="pv0")
                for kc in range(16):
                    xtt = xts[:, kc, tsl]
                    st, sp = kc == 0, kc == 15
                    nc.tensor.matmul(pk, wq_t[:, kc, 512:640], xtt, start=st, stop=sp)
                    nc.tensor.matmul(pv, wq_t[:, kc, 640:768], xtt, start=st, stop=sp)
                    nc.tensor.matmul(pQ[:, 0:512], wq_t[:, kc, 0:128], xtt, start=st, stop=sp)
                    nc.tensor.matmul(pQ[:, 512:1024], wq_t[:, kc, 128:256], xtt, start=st, stop=sp)
                    nc.tensor.matmul(pR[:, 0:512], wq_t[:, kc, 256:384], xtt, start=st, stop=sp)
                    nc.tensor.matmul(pR[:, 512:1024], wq_t[:, kc, 384:512], xtt, start=st, stop=sp)
            else:
                # chunks 1-3: A-pass (k/v) was queued as filler during the
                # previous section; finish whatever remains
                pk, pv, _ = kvt
                drain_filler(len(filler))
            nc.scalar.copy(out=qk[4][:, tsl], in_=pk)
            vtt = vttp.tile([128, 512], MD, tag="vtt", name=f"vtt{tcn}")
            nc.vector.tensor_copy(out=vtt, in_=pv)
            if tcn == 0:  # chunk-0 k feeds the c0 segments this phase;
                # later chunks' k-rope can wait until after them
                rope(4, tcn)
            if tcn > 0:
                # B-pass: q projections
                for kc in range(16):
                    xtt = xts[:, kc, tsl]
                    st, sp = kc == 0, kc == 15
                    nc.tensor.matmul(pQ[:, 0:512], wq_t[:, kc, 0:128], xtt, start=st, stop=sp)
                    nc.tensor.matmul(pQ[:, 512:1024], wq_t[:, kc, 128:256], xtt, start=st, stop=sp)
                    nc.tensor.matmul(pR[:, 0:512], wq_t[:, kc, 256:384], xtt, start=st, stop=sp)
                    nc.tensor.matmul(pR[:, 512:1024], wq_t[:, kc, 384:512], xtt, start=st, stop=sp)
            nc.scalar.copy(out=qk[0][:, tsl], in_=pQ[:, 0:512])
            nc.vector.tensor_copy(out=qk[1][:, tsl], in_=pQ[:, 512:1024])
            nc.scalar.copy(out=qk[2][:, tsl], in_=pR[:, 0:512])
            nc.vector.tensor_copy(out=qk[3][:, tsl], in_=pR[:, 512:1024])
            g, r0 = tcn // 2, (tcn % 2) * 4
            if tcn == 0:
                # chunk-0 v feeds attn(0,0) next section — issue now
                nc.scalar.dma_start_transpose(
                    out=vsb[g][:, r0 : r0 + 4, :], in_=vtt
                )
            if tcn < 3:  # next chunk's A-pass becomes tensor filler
                kvt = make_a_steps(tcn + 1)
                filler.extend(kvt[2])
            # c0 attention before this chunk's q-rope: the segments only
            # need chunk-0 data, and the rope's ~7us of vector work (not
            # needed until the weave bands) would otherwise delay their
            # masks/denominator chain on the vector engine
            for c0h in attn_after.get(tcn, []):
                attn_segment(0, c0h)
            for o in ([0, 1, 2, 3] if tcn == 0 else [4, 0, 1, 2, 3]):
                rope(o, tcn)
            if tcn > 0:
                # chunks 1-3's transposed v is only needed by the weave
                # bands — issue at section end so the transpose's long
                # queue-semaphore pre-wait never parks the scalar engine
                # ahead of the evac copies
                nc.scalar.dma_start_transpose(
                    out=vsb[g][:, r0 : r0 + 4, :], in_=vtt
                )
            drain_filler(4)

        for c in range(1, 4):
            band_ms = range(4 * (c - 1), 4 * (c - 1) + 4)
            filler.extend(
                make_quarter(m, cn) for m in band_ms for cn in range(4)
            )
            for h in range(4):
                attn_segment(c, h)
                # pace: by the end of segment h, 4*(h+1) quarters should be out
                done = 16 - len(filler)
                drain_filler(4 * (h + 1) - done)
            drain_filler(len(filler))
        for m in range(12, 16):
            p3_mgroup(m, last=True)

    nc.compile()
    return nc


_PERM = np.concatenate([np.arange(0, D, 2), np.arange(1, D, 2)])

import ml_dtypes

DT_NP = ml_dtypes.bfloat16


def make_in_maps(x, freqs_cos, freqs_sin, Wq, Wk, Wv, Wo):
    x = np.asarray(x, dtype=np.float32)
    freqs_cos = np.asarray(freqs_cos, dtype=np.float32)
    freqs_sin = np.asarray(freqs_sin, dtype=np.float32)
    Wq = np.asarray(Wq, dtype=np.float32)
    Wk = np.asarray(Wk, dtype=np.float32)
    Wv = np.asarray(Wv, dtype=np.float32)
    Wo = np.asarray(Wo, dtype=np.float32)

    scale = 1.0 / np.sqrt(np.float32(D))
    cosT = np.ascontiguousarray(freqs_cos.T)
    sinT = np.ascontiguousarray(freqs_sin.T)
    cc = np.concatenate([cosT, cosT], axis=0)  # [128, T]
    ss = np.concatenate([-sinT, sinT], axis=0)  # [128, T]
    css = np.ascontiguousarray(np.concatenate([cc, ss], axis=1)).astype(DT_NP)
    wk_p = Wk[:, _PERM]  # [C, 128]
    # [128, 16, cols] partition-major weight blocks
    wk_b = wk_p.reshape(16, 128, 128).transpose(1, 0, 2)
    wv_b = Wv.reshape(16, 128, 128).transpose(1, 0, 2)

    xts = [np.ascontiguousarray(x[b].T).astype(DT_NP) for b in range(2)]

    in_maps = []
    for core in range(N_CORES):
        b = core // 4
        hg = core % 4
        heads = range(4 * hg, 4 * hg + 4)
        qcols = np.concatenate([h * D + _PERM for h in heads])
        wq_c = (Wq[:, qcols] * scale).reshape(16, 128, 512).transpose(1, 0, 2)
        wqkv = np.ascontiguousarray(
            np.concatenate([wq_c, wk_b, wv_b], axis=2)
        ).astype(DT_NP)
        orows = np.concatenate([np.arange(h * D, (h + 1) * D) for h in heads])
        wo_c = np.ascontiguousarray(
            Wo[orows, :].reshape(4, 128, 2048).transpose(1, 0, 2)
        ).astype(DT_NP)
        in_maps.append(
            {
                "xt": xts[b],
                "wqkv": wqkv,
                "wo": wo_c,
                "css": css,
            }
        )
    return in_maps


_PROGRAM = None


def get_program():
    global _PROGRAM
    if _PROGRAM is None:
        _PROGRAM = build_program()
    return _PROGRAM


def kernel(x, freqs_cos, freqs_sin, Wq, Wk, Wv, Wo, _collect=None):
    nc = get_program()
    in_maps = make_in_maps(x, freqs_cos, freqs_sin, Wq, Wk, Wv, Wo)
    res = run_bass_kernel_spmd(nc, in_maps, core_ids=list(range(N_CORES)))
    if _collect is not None:
        _collect.append(res)
    outs = [
        np.asarray(r["out"], dtype=np.float32).transpose(1, 0, 2).reshape(T, C)
        for r in res.results
    ]
    full = np.empty((2, T, C), dtype=np.float32)
    for b in range(2):
        full[b] = outs[4 * b] + outs[4 * b + 1] + outs[4 * b + 2] + outs[4 * b + 3]
    return full


# revision 62
# speedup vs baseline: 1.0301x; 1.0301x over previous
"""MQA attention kernel v26 (B=2, T=2048, C=2048, 16 query heads, D=128,
RoPE, causal) for 8 Trainium2 NeuronCores.

Sharding: core = (batch, head-group-of-4), partial output projections summed
on host.  ~253us median / ~251us best at the fast clock state, ~290us slow
state (vs 283/338us v4 baseline); rel err 6.4e-3.

v26: weight quarters q2/q3 ride the sync queue interleaved with the x
pairs in kc-need order, so the 3MB weight stream no longer trails the x
stream (chunk-0 window idle 8.2 -> 2.7us).

v22..24 theme — on in-order engines, emission position IS the schedule; emit
deferrable work last:
- c0 attention segments run BEFORE each section's rope work (the ~7us of
  rope vector-ops, not needed until the weave bands, were delaying the
  segments' mask/denominator chain); k-rope of chunks 1-3 deferred too.
- the v-transpose DMA issues AFTER the q-evac copies: its semaphore
  pre-wait otherwise parks the scalar engine ~8us between the k-evac and
  the q-evacs that gate the next section's B-pass PSUM banks.

v20 key fix: dma_start BLOCKS the issuing engine until the previous
transfer on the same queue-semaphore completes — so all x^T waves live on
the otherwise-idle sync queue (its in-order blocking IS the demand pacing)
and the scalar engine keeps only the 4 weight-quarter issues, leaving it
free for the PSUM evac copies that gate each phase-1 section boundary.

Structure (v5..v19 over the v4 baseline):
- DMA (v5..v19): host pre-arranges all tensors partition-major (>=1KB contiguous
  lines); x^T fully SBUF-resident via large demand-ordered DMAs (weight
  quarters interleaved with x t-chunk-0 columns in kc order, then chunk 1,
  then t-half B); fused wq|wk|wv dram tensor; Wo loaded at tcn=2; outputs
  staged per 128-row group, large row DMAs on rotating queues, per-quarter
  drain for the last 4 groups (gpsimd excluded there so its ~5us engine
  drain retires early).
- PE warmup matmuls on a memset tile cover the ~7us framework preamble +
  first input DMAs with the clock fully ramped.
- Unified tensor-filler list: the in-order PE queue never sits behind an
  exp-dependent matmul — output-projection [128,512] po-quarters (band
  c uses chunk c-1's m-groups) and the NEXT chunk's k/v kc-steps (A-pass)
  are drained between score pairs / after diagonal scores.  Phase-1 chunk 0
  keeps the dense 6-stream loop (it is DMA-paced).
- attn segments: 2-pair softmax lookahead (scores(p+2) before pv(p));
  denominator ones-matmuls deferred past the DVE add tree; v transposed via
  dma_start_transpose straight into the [keys,D] SBUF layout (no PSUM /
  tensor-engine transposes).
Known floor: ~203us of bf16 matmul rows at 2.4GHz; fp8 DoubleRow and
AllGather k/v-dedup were measured and ruled out (see memory notes).
"""

import os
import sys

if "/opt/trn_rl_repo" not in sys.path:
    sys.path.insert(0, "/opt/trn_rl_repo")

import numpy as np

import concourse.bacc as bacc
import concourse.mybir as mybir
import concourse.tile as tile
from concourse.bass_utils import run_bass_kernel_spmd

T = 2048
C = 2048
D = 128
N_HEAD = 16
HPC = 4
N_CORES = 8
F32 = mybir.dt.float32
BF16 = mybir.dt.bfloat16
EXP = mybir.ActivationFunctionType.Exp

MD = BF16


def build_program():
    nc = bacc.Bacc("TRN2", target_bir_lowering=False, debug=False)

    xt = nc.dram_tensor("xt", [C, T], MD, kind="ExternalInput")
    wqkv = nc.dram_tensor("wqkv", [128, 16, 768], MD, kind="ExternalInput")
    wo = nc.dram_tensor("wo", [128, 4, 2048], MD, kind="ExternalInput")
    css = nc.dram_tensor("css", [128, 2 * T], MD, kind="ExternalInput")
    out = nc.dram_tensor("out", [128, 16, 2048], BF16, kind="ExternalOutput")

    xt_r = xt.rearrange("(ko p) t -> p ko t", p=128)

    with (
        tile.TileContext(nc) as tc,
        tc.tile_pool(name="consts", bufs=1) as consts,
        tc.tile_pool(name="qkpool", bufs=5) as qkpool,
        tc.tile_pool(name="ytpool", bufs=4) as ytpool,
        tc.tile_pool(name="vttp", bufs=2) as vttp,
        tc.tile_pool(name="ptp", bufs=5) as ptpool,
        tc.tile_pool(name="otp", bufs=3) as otp,
        tc.tile_pool(name="swp", bufs=3) as swp,
        tc.tile_pool(name="pad", bufs=6) as pad,
        tc.tile_pool(name="bcp", bufs=3) as bcp,
        tc.tile_pool(name="psb", bufs=2, space="PSUM") as psb,
        tc.tile_pool(name="pss", bufs=4, space="PSUM") as pssm,
    ):
        # ---- input DMAs: big contiguous-line transfers, demand-ordered so
        # the ~315GB/s per-core DMA bandwidth goes to what compute needs
        # next.  Upfront: sync queue x^T t-half A (chunks 0-1), scalar queue
        # weight quarters, gpsimd cos/sin.  x^T t-half B is issued at tcn=1,
        # Wo at tcn=2 (see phase 1 loop). ----
        # PE warmup: dependency-free matmuls on a memset tile keep the PE
        # busy through the DMA wait so the clock is fully ramped (and the
        # pipeline hot) when the real accumulation chains start.
        warm_mv = consts.tile([128, 512], MD, tag="warm")
        nc.gpsimd.memset(warm_mv, 0.0)

        # input DMAs: interleave weight quarters and x^T pairs across the
        # sync/scalar queues so arrival order tracks the kc consumption
        # order of the first t-chunk.
        xts = consts.tile([128, 16, T], MD, tag="xts")
        wq_t = consts.tile([128, 16, 768], MD, tag="wq")

        def wq_dma(eng, q):
            eng.dma_start(
                out=wq_t[:, 4 * q : 4 * q + 4, :], in_=wqkv[:, 4 * q : 4 * q + 4, :]
            )

        def xt_dma(eng, kp, half):
            tsl = slice(0, 512)
            eng.dma_start(
                out=xts[:, 2 * kp : 2 * kp + 2, tsl],
                in_=xt_r[:, 2 * kp : 2 * kp + 2, tsl],
            )

        csst = consts.tile([128, 2 * T], MD, tag="css")
        nc.gpsimd.dma_start(out=csst, in_=css[:, :])
        # wave 0: weight quarters interleaved with t-chunk-0 columns only,
        # in kc order — chunk 0's consumption is DMA-paced, so nothing else
        # competes for bandwidth until its last kc tile has landed
        # x^T entirely on the sync queue: dma_start blocks the issuing
        # engine until the previous transfer on the same semaphore is done,
        # so the idle sync engine absorbs all the pacing stalls while the
        # scalar engine stays free for PSUM evac copies.
        # weight quarters q0/q1 on scalar; q2/q3 interleaved into the sync
        # queue in kc-need order — otherwise the 3MB weight stream trails
        # the 2MB x stream and q3's arrival (~26us) gates kc12-15
        wq_dma(nc.scalar, 0)
        wq_dma(nc.scalar, 1)
        for kp in range(4):
            xt_dma(nc.sync, kp, 0)
        wq_dma(nc.sync, 2)
        xt_dma(nc.sync, 4, 0)
        xt_dma(nc.sync, 5, 0)
        wq_dma(nc.sync, 3)
        xt_dma(nc.sync, 6, 0)
        xt_dma(nc.sync, 7, 0)
        for kp in range(8):
            nc.sync.dma_start(
                out=xts[:, 2 * kp : 2 * kp + 2, 512:1024],
                in_=xt_r[:, 2 * kp : 2 * kp + 2, 512:1024],
            )
        for kp in range(8):
            nc.sync.dma_start(
                out=xts[:, 2 * kp : 2 * kp + 2, 1024:2048],
                in_=xt_r[:, 2 * kp : 2 * kp + 2, 1024:2048],
            )
        for w in range(24):
            pw = pssm.tile([128, 512], F32, tag="small", name=f"warm{w}")
            nc.tensor.matmul(pw, warm_mv[:, 0:128], warm_mv, start=True, stop=True)

        wot = consts.tile([128, 4, 2048], MD, tag="wo")

        # on-chip constants: ones / causal-triangle
        ones = consts.tile([128, 128], MD, tag="ones")
        nc.gpsimd.memset(ones, 1.0)
        tri = consts.tile([128, 128], MD, tag="tri")
        nc.gpsimd.memset(tri, 1.0)
        nc.gpsimd.affine_select(
            out=tri,
            in_=tri,
            compare_op=mybir.AluOpType.is_ge,
            fill=0.0,
            base=0,
            pattern=[[1, 128]],
            channel_multiplier=-1,
        )

        qk = [qkpool.tile([128, T], MD, tag="qk", name=f"qk{i}") for i in range(5)]
        yt = [ytpool.tile([128, T], MD, tag="yt", name=f"yt{h}") for h in range(4)]
        vsb = [consts.tile([128, 8, 128], MD, tag=f"vsb{g}", name=f"vsb{g}") for g in range(2)]

        def vtile(j):
            return vsb[j // 8][:, j % 8, :]

        def wosl(h, cn):  # [128, 512] slice of Wo for output cols cn
            return wot[:, h, cn * 512 : (cn + 1) * 512]

        def ktile(j):
            return qk[4][:, j * 128 : (j + 1) * 128]

        # ---- phase 2 + 3 woven: attention per (chunk, head); the previous
        # chunk's output-projection emits as [128,512] po-quarter filler
        # INSIDE each segment (between score pairs), so the in-order tensor
        # queue always has exp-independent work while softmax runs ----
        filler = []

        def drain_filler(n):
            while n > 0 and filler:
                filler.pop(0)()
                n -= 1

        def attn_segment(c, h):
            if c == 0:
                # c0 segments have no pair loop: without this, their
                # diagonal scores (which wait on the pQ/pR bank evacs) sit
                # at the head of the tensor queue with no filler ahead.
                # 4 steps only: deeper drains stall at the queue head on
                # not-yet-landed t-half-B columns (measured regression).
                drain_filler(4)
            qsl = qk[h][:, c * 512 : (c + 1) * 512]
            py = pssm.tile([128, 512], F32, tag="small", name=f"py{c}_{h}")
            psm = pssm.tile([128, 512], F32, tag="small", name=f"psm{c}_{h}")
            py_on = False
            sm_on = False
            pend = None
            pend2 = None

            def emit_pv(pT, j0):
                nonlocal py_on
                nc.tensor.matmul(py, vtile(j0), pT[:, 0:512], start=not py_on, stop=False)
                py_on = True
                nc.tensor.matmul(py, vtile(j0 + 1), pT[:, 512:1024], start=False, stop=False)

            sm_src = []  # summed-exp tiles; their ones-matmuls are deferred
            # to the segment end so the tensor queue never stalls on the
            # DVE add tree.
            pvq = []  # two-pair lookahead: pv(p) is emitted after
            # scores(p+2), giving each exp ~2 score-pairs of tensor cover
            for p in range(2 * c):
                j0 = 2 * p
                pss = psb.tile([128, 1024], F32, tag="big", name=f"pss{c}_{h}_{p}")
                nc.tensor.matmul(pss[:, 0:512], ktile(j0), qsl, start=True, stop=True)
                nc.tensor.matmul(pss[:, 512:1024], ktile(j0 + 1), qsl, start=True, stop=True)
                if len(pvq) >= 2:
                    emit_pv(*pvq.pop(0))
                drain_filler(3 - c if c < 3 else 1)
                pT = ptpool.tile([128, 1024], MD, tag="pt", name=f"pt{c}_{h}_{p}")
                nc.scalar.activation(out=pT, in_=pss, func=EXP)
                pvq.append((pT, j0))
                padd = pad.tile([128, 512], MD, tag="padd", name=f"pa{c}_{h}_{p}")
                nc.vector.tensor_add(out=padd, in0=pT[:, 0:512], in1=pT[:, 512:1024])
                if pend is None:
                    pend = padd
                else:
                    qadd = pad.tile([128, 512], MD, tag="padd", name=f"qa{c}_{h}_{p}")
                    nc.vector.tensor_add(out=qadd, in0=pend, in1=padd)
                    pend = None
                    if pend2 is None:
                        pend2 = qadd
                    else:
                        oadd = pad.tile([128, 512], MD, tag="padd", name=f"oa{c}_{h}_{p}")
                        nc.vector.tensor_add(out=oadd, in0=pend2, in1=qadd)
                        sm_src.append(oadd)
                        pend2 = None
            if pend2 is not None:
                sm_src.append(pend2)
                pend2 = None
            # diagonal group: r0 [0:512] + r1 [512:896] in A; r2 [0:256] +
            # r3 [256:384] in B (both allocated up front: no exp stall)
            jb = 4 * c
            pdA = psb.tile([128, 1024], F32, tag="big", name=f"pdA{c}_{h}")
            pdB = psb.tile([128, 1024], F32, tag="big", name=f"pdB{c}_{h}")
            nc.tensor.matmul(pdA[:, 0:512], ktile(jb), qsl, start=True, stop=True)
            nc.tensor.matmul(
                pdA[:, 512:896],
                ktile(jb + 1),
                qk[h][:, c * 512 + 128 : (c + 1) * 512],
                start=True,
                stop=True,
            )
            nc.tensor.matmul(
                pdB[:, 0:256],
                ktile(jb + 2),
                qk[h][:, c * 512 + 256 : (c + 1) * 512],
                start=True,
                stop=True,
            )
            nc.tensor.matmul(
                pdB[:, 256:384],
                ktile(jb + 3),
                qk[h][:, c * 512 + 384 : (c + 1) * 512],
                start=True,
                stop=True,
            )
            while pvq:
                emit_pv(*pvq.pop(0))
            for oadd in sm_src:
                nc.tensor.matmul(psm, ones, oadd, start=not sm_on, stop=False)
                sm_on = True
            drain_filler(2)
            pTA = ptpool.tile([128, 1024], MD, tag="pt", name=f"ptA{c}_{h}")
            pTB = ptpool.tile([128, 1024], MD, tag="pt", name=f"ptB{c}_{h}")
            nc.scalar.activation(out=pTA[:, 0:896], in_=pdA[:, 0:896], func=EXP)
            nc.scalar.activation(out=pTB[:, 0:384], in_=pdB[:, 0:384], func=EXP)
            ve = nc.vector
            ve.tensor_mul(out=pTA[:, 0:128], in0=pTA[:, 0:128], in1=tri)
            ve.tensor_mul(out=pTA[:, 512:640], in0=pTA[:, 512:640], in1=tri)
            ve.tensor_mul(out=pTB[:, 0:128], in0=pTB[:, 0:128], in1=tri)
            ve.tensor_mul(out=pTB[:, 256:384], in0=pTB[:, 256:384], in1=tri)
            nc.tensor.matmul(py, vtile(jb), pTA[:, 0:512], start=not py_on, stop=False)
            nc.tensor.matmul(py[:, 128:512], vtile(jb + 1), pTA[:, 512:896], start=False, stop=False)
            nc.tensor.matmul(py[:, 256:512], vtile(jb + 2), pTB[:, 0:256], start=False, stop=False)
            nc.tensor.matmul(py[:, 384:512], vtile(jb + 3), pTB[:, 256:384], start=False, stop=True)
            # diagonal denominators collapse on DVE, then one ones-matmul
            pd = pad.tile([128, 512], MD, tag="padd", name=f"pd{c}_{h}")
            ve.tensor_copy(out=pd[:, 0:128], in_=pTA[:, 0:128])
            ve.tensor_add(out=pd[:, 128:512], in0=pTA[:, 128:512], in1=pTA[:, 512:896])
            ve.tensor_add(out=pd[:, 256:512], in0=pd[:, 256:512], in1=pTB[:, 0:256])
            ve.tensor_add(out=pd[:, 384:512], in0=pd[:, 384:512], in1=pTB[:, 256:384])
            nc.tensor.matmul(psm, ones, pd, start=not sm_on, stop=True)
            bc = bcp.tile([128, 512], F32, tag="bc", name=f"bc{c}_{h}")
            nc.vector.reciprocal_approx_fast(out=bc, in_=psm)
            nc.vector.tensor_mul(
                out=yt[h][:, c * 512 : (c + 1) * 512], in0=py, in1=bc
            )

        out_q = [nc.sync, nc.gpsimd, nc.scalar]
        otms = {}

        def make_quarter(m, cn, last=False):
            def q():
                if cn == 0:
                    otms[m] = otp.tile([128, 2048], MD, tag="ot", name=f"ot{m}")
                otm = otms[m]
                po = pssm.tile([128, 512], F32, tag="small", name=f"po{m}_{cn}")
                for h in range(4):
                    nc.tensor.matmul(
                        po,
                        yt[h][:, m * 128 : (m + 1) * 128],
                        wosl(h, cn),
                        start=h == 0,
                        stop=h == 3,
                    )
                osl = otm[:, cn * 512 : (cn + 1) * 512]
                if cn == 0 or cn == 2:
                    nc.vector.tensor_copy(out=osl, in_=po)
                else:
                    nc.scalar.copy(out=osl, in_=po)
                if last:  # drain each quarter immediately; avoid gpsimd so
                    # its ~5us engine drain retires before the last compute
                    (nc.sync if (m + cn) % 2 else nc.scalar).dma_start(
                        out=out[:, m, cn * 512 : (cn + 1) * 512], in_=osl
                    )
                elif cn == 3:
                    out_q[m % 3].dma_start(out=out[:, m, :], in_=otm)
                if cn == 3:
                    del otms[m]
            return q

        def p3_mgroup(m, last=False):
            for cn in range(4):
                make_quarter(m, cn, last)()

        # ---- phase 1: q/k/v projections, t-chunk-major, split per chunk
        # into an A-pass (k/v) and a B-pass (q).  The NEXT chunk's A-pass
        # kc-steps are queued as filler so the c0 attention segments woven
        # into each chunk's tail never leave the tensor engine idle. ----
        attn_after = {1: [0], 2: [1, 2], 3: [3]}  # tcn -> c0 heads to emit

        def rope(o, tcn):
            tsl = slice(512 * tcn, 512 * (tcn + 1))
            qc = qk[o]
            sw = swp.tile([128, 512], MD, tag="sw", name=f"sw{tcn}_{o}")
            nc.gpsimd.dma_start(out=sw[0:64, :], in_=qc[64:128, tsl])
            nc.gpsimd.dma_start(out=sw[64:128, :], in_=qc[0:64, tsl])
            nc.vector.tensor_mul(out=qc[:, tsl], in0=qc[:, tsl], in1=csst[:, tsl])
            eng = nc.gpsimd if o in (1, 2) else nc.vector
            eng.tensor_mul(
                out=sw[:], in0=sw[:], in1=csst[:, T + 512 * tcn : T + 512 * (tcn + 1)]
            )
            nc.vector.tensor_add(out=qc[:, tsl], in0=qc[:, tsl], in1=sw[:])

        def make_a_steps(tcn):
            tsl = slice(512 * tcn, 512 * (tcn + 1))
            pk = pssm.tile([128, 512], F32, tag="small", name=f"pk{tcn}")
            pv = pssm.tile([128, 512], F32, tag="small", name=f"pv{tcn}")

            def step(kc):
                def f():
                    xtt = xts[:, kc, tsl]
                    st, sp = kc == 0, kc == 15
                    nc.tensor.matmul(pk, wq_t[:, kc, 512:640], xtt, start=st, stop=sp)
                    nc.tensor.matmul(pv, wq_t[:, kc, 640:768], xtt, start=st, stop=sp)
                return f

            return pk, pv, [step(kc) for kc in range(16)]

        kvt = None
        for tcn in range(4):
            if tcn == 2:  # Wo: needed from the first p3_mgroup
                nc.sync.dma_start(out=wot[:, :, 0:1024], in_=wo[:, :, 0:1024])
                nc.sync.dma_start(out=wot[:, :, 1024:2048], in_=wo[:, :, 1024:2048])
            tsl = slice(512 * tcn, 512 * (tcn + 1))
            pQ = psb.tile([128, 1024], F32, tag="big", name=f"pQ{tcn}")  # q0|q1
            pR = psb.tile([128, 1024], F32, tag="big", name=f"pR{tcn}")  # q2|q3
            if tcn == 0:
                # chunk 0 is DMA-paced: keep the dense combined loop so
                # every arriving kc tile feeds 6 matmuls at once
                pk = pssm.tile([128, 512], F32, tag="small", name="pk0")
                pv = pssm.tile([128, 512], F32, tag="small", name="pv0")
                for kc in range(16):
                    xtt = xts[:, kc, tsl]
                    st, sp = kc == 0, kc == 15
                    nc.tensor.matmul(pk, wq_t[:, kc, 512:640], xtt, start=st, stop=sp)
                    nc.tensor.matmul(pv, wq_t[:, kc, 640:768], xtt, start=st, stop=sp)
                    nc.tensor.matmul(pQ[:, 0:512], wq_t[:, kc, 0:128], xtt, start=st, stop=sp)
                    nc.tensor.matmul(pQ[:, 512:1024], wq_t[:, kc, 128:256], xtt, start=st, stop=sp)
                    nc.tensor.matmul(pR[:, 0:512], wq_t[:, kc, 256:384], xtt, start=st, stop=sp)
                    nc.tensor.matmul(pR[:, 512:1024], wq_t[:, kc, 384:512], xtt, start=st, stop=sp)
            else:
                # chunks 1-3: A-pass (k/v) was queued as filler during the
                # previous section; finish whatever remains
                pk, pv, _ = kvt
                drain_filler(len(filler))
            nc.scalar.copy(out=qk[4][:, tsl], in_=pk)
            vtt = vttp.tile([128, 512], MD, tag="vtt", name=f"vtt{tcn}")
            nc.vector.tensor_copy(out=vtt, in_=pv)
            if tcn == 0:  # chunk-0 k feeds the c0 segments this phase;
                # later chunks' k-rope can wait until after them
                rope(4, tcn)
            if tcn > 0:
                # B-pass: q projections
                for kc in range(16):
                    xtt = xts[:, kc, tsl]
                    st, sp = kc == 0, kc == 15
                    nc.tensor.matmul(pQ[:, 0:512], wq_t[:, kc, 0:128], xtt, start=st, stop=sp)
                    nc.tensor.matmul(pQ[:, 512:1024], wq_t[:, kc, 128:256], xtt, start=st, stop=sp)
                    nc.tensor.matmul(pR[:, 0:512], wq_t[:, kc, 256:384], xtt, start=st, stop=sp)
                    nc.tensor.matmul(pR[:, 512:1024], wq_t[:, kc, 384:512], xtt, start=st, stop=sp)
            nc.scalar.copy(out=qk[0][:, tsl], in_=pQ[:, 0:512])
            nc.vector.tensor_copy(out=qk[1][:, tsl], in_=pQ[:, 512:1024])
            nc.scalar.copy(out=qk[2][:, tsl], in_=pR[:, 0:512])
            nc.vector.tensor_copy(out=qk[3][:, tsl], in_=pR[:, 512:1024])
            g, r0 = tcn // 2, (tcn % 2) * 4
            if tcn == 0:
                # chunk-0 v feeds attn(0,0) next section — issue now
                nc.scalar.dma_start_transpose(
                    out=vsb[g][:, r0 : r0 + 4, :], in_=vtt
                )
            if tcn < 3:  # next chunk's A-pass becomes tensor filler
                kvt = make_a_steps(tcn + 1)
                filler.extend(kvt[2])
            # c0 attention before this chunk's q-rope: the segments only
            # need chunk-0 data, and the rope's ~7us of vector work (not
            # needed until the weave bands) would otherwise delay their
            # masks/denominator chain on the vector engine
            for c0h in attn_after.get(tcn, []):
                attn_segment(0, c0h)
            for o in ([0, 1, 2, 3] if tcn == 0 else [4, 0, 1, 2, 3]):
                rope(o, tcn)
            if tcn > 0:
                # chunks 1-3's transposed v is only needed by the weave
                # bands — issue at section end so the transpose's long
                # queue-semaphore pre-wait never parks the scalar engine
                # ahead of the evac copies
                nc.scalar.dma_start_transpose(
                    out=vsb[g][:, r0 : r0 + 4, :], in_=vtt
                )
            drain_filler(4)

        for c in range(1, 4):
            band_ms = range(4 * (c - 1), 4 * (c - 1) + 4)
            filler.extend(
                make_quarter(m, cn) for m in band_ms for cn in range(4)
            )
            for h in range(4):
                attn_segment(c, h)
                # pace: by the end of segment h, 4*(h+1) quarters should be out
                done = 16 - len(filler)
                drain_filler(4 * (h + 1) - done)
            drain_filler(len(filler))
        for m in range(12, 16):
            p3_mgroup(m, last=True)

    nc.compile()
    return nc


_PERM = np.concatenate([np.arange(0, D, 2), np.arange(1, D, 2)])

import ml_dtypes

DT_NP = ml_dtypes.bfloat16


def make_in_maps(x, freqs_cos, freqs_sin, Wq, Wk, Wv, Wo):
    x = np.asarray(x, dtype=np.float32)
    freqs_cos = np.asarray(freqs_cos, dtype=np.float32)
    freqs_sin = np.asarray(freqs_sin, dtype=np.float32)
    Wq = np.asarray(Wq, dtype=np.float32)
    Wk = np.asarray(Wk, dtype=np.float32)
    Wv = np.asarray(Wv, dtype=np.float32)
    Wo = np.asarray(Wo, dtype=np.float32)

    scale = 1.0 / np.sqrt(np.float32(D))
    cosT = np.ascontiguousarray(freqs_cos.T)
    sinT = np.ascontiguousarray(freqs_sin.T)
    cc = np.concatenate([cosT, cosT], axis=0)  # [128, T]
    ss = np.concatenate([-sinT, sinT], axis=0)  # [128, T]
    css = np.ascontiguousarray(np.concatenate([cc, ss], axis=1)).astype(DT_NP)
    wk_p = Wk[:, _PERM]  # [C, 128]
    # [128, 16, cols] partition-major weight blocks
    wk_b = wk_p.reshape(16, 128, 128).transpose(1, 0, 2)
    wv_b = Wv.reshape(16, 128, 128).transpose(1, 0, 2)

    xts = [np.ascontiguousarray(x[b].T).astype(DT_NP) for b in range(2)]

    in_maps = []
    for core in range(N_CORES):
        b = core // 4
        hg = core % 4
        heads = range(4 * hg, 4 * hg + 4)
        qcols = np.concatenate([h * D + _PERM for h in heads])
        wq_c = (Wq[:, qcols] * scale).reshape(16, 128, 512).transpose(1, 0, 2)
        wqkv = np.ascontiguousarray(
            np.concatenate([wq_c, wk_b, wv_b], axis=2)
        ).astype(DT_NP)
        orows = np.concatenate([np.arange(h * D, (h + 1) * D) for h in heads])
        wo_c = np.ascontiguousarray(
            Wo[orows, :].reshape(4, 128, 2048).transpose(1, 0, 2)
        ).astype(DT_NP)
        in_maps.append(
            {
                "xt": xts[b],
                "wqkv": wqkv,
                "wo": wo_c,
                "css": css,
            }
        )
    return in_maps


_PROGRAM = None


def get_program():
    global _PROGRAM
    if _PROGRAM is None:
        _PROGRAM = build_program()
    return _PROGRAM


def kernel(x, freqs_cos, freqs_sin, Wq, Wk, Wv, Wo, _collect=None):
    nc = get_program()
    in_maps = make_in_maps(x, freqs_cos, freqs_sin, Wq, Wk, Wv, Wo)
    res = run_bass_kernel_spmd(nc, in_maps, core_ids=list(range(N_CORES)))
    if _collect is not None:
        _collect.append(res)
    outs = [
        np.asarray(r["out"], dtype=np.float32).transpose(1, 0, 2).reshape(T, C)
        for r in res.results
    ]
    full = np.empty((2, T, C), dtype=np.float32)
    for b in range(2):
        full[b] = outs[4 * b] + outs[4 * b + 1] + outs[4 * b + 2] + outs[4 * b + 3]
    return full
